# revision 21
# baseline (speedup 1.0000x reference)
"""Chamfer loss on 8 Trainium2 NeuronCores (Bass/Tile).

Problem: gts [16,4096,3] f32, preds [16,4096,3] f32 ->
  loss = mean(min_n ||g_n - p_m||^2) + mean(min_m ||g_n - p_m||^2)  (scalar f32)

Strategy (hardcoded shapes, data-parallel over batch: 2 batches per core):
  * Compute NEGATED squared distances S = 2 g.p - |g|^2 - |p|^2 (= -dist^2)
    with a single K=13 fp16 hi/lo-split augmented matmul per 128x512 tile:
    full fp32-class accuracy at bf16 matmul speed (1 cycle/row). All mins
    become maxes (max ops are what DVE offers everywhere).
  * Per n-tile (128 gts points): 8 matmuls fill 2x4 PSUM banks; ScalarE
    evicts them fp32->fp16 into one t16 [128,4096] SBUF tile; VectorE does
      - col path: colacc = max(colacc, t16)    (one full-width TT, 2x mode)
      - row path: fresh max-tree 4096 -> 512 into a per-tile h3 store
    (tensor_tensor_reduce would fuse the row path into one op but fails in
    the HW toolchain - bisected 2026-08-04.)
  * Batch end: fold rowh3s [128,32x512] -> rowcon [128,32] on DVE.
  * colacc [128,4096] fp16 and rowcon [128,32] f32 are DMA'd out per batch;
    the tiny final folds (max over 128 partitions, mean, negate) run on host.
Measured ~290-315us HW on 8 cores (DVE-bound; modeled DVE busy 285us).
"""

import numpy as np
from contextlib import ExitStack

N_CORES = 8
B, N, M, D = 16, 4096, 4096, 3
BPC = B // N_CORES          # batches per core
NT = N // 128               # 32 n-tiles
MBLK = 512                  # m-block (one PSUM bank of fp32)
MB = M // MBLK              # 8 m-blocks
G = 4                       # m-blocks per group (4 banks evicted at once)
NG = MB // G                # 2 groups
K = 13                      # augmented contraction dim

_CACHE = {}


def _build_nc(repeat=None):
    from concourse import bacc, mybir, tile

    F32 = mybir.dt.float32
    F16 = mybir.dt.float16
    mx = mybir.AluOpType.max

    nc = bacc.Bacc("TRN2", target_bir_lowering=False, debug=False,
                   num_devices=N_CORES)

    la = nc.dram_tensor("la", [BPC, K, N], F16, kind="ExternalInput").ap()
    ra = nc.dram_tensor("ra", [BPC, K, M], F16, kind="ExternalInput").ap()
    colaccs = nc.dram_tensor("colaccs", [BPC, 128, M], F16,
                             kind="ExternalOutput").ap()
    rowcons = nc.dram_tensor("rowcons", [BPC, 128, NT], F32,
                             kind="ExternalOutput").ap()

    with tile.TileContext(nc) as tc, ExitStack() as ctx:
        aug = ctx.enter_context(tc.tile_pool(name="aug", bufs=2))
        ps = ctx.enter_context(tc.tile_pool(name="ps", bufs=2, space="PSUM"))
        evp = ctx.enter_context(tc.tile_pool(name="ev", bufs=3))
        tre = ctx.enter_context(tc.tile_pool(name="tre", bufs=2))
        accp = ctx.enter_context(tc.tile_pool(name="acc", bufs=2))
        rowp = ctx.enter_context(tc.tile_pool(name="rowp", bufs=1))

        if repeat is not None:
            rep_cm = tc.For_i(0, repeat, 1)
            rep_cm.__enter__()

        for b in range(BPC):
            la_sb = aug.tile([K, N], F16, tag="la")
            ra_sb = aug.tile([K, M], F16, tag="ra")
            nc.sync.dma_start(la_sb[:], la[b])
            nc.sync.dma_start(ra_sb[:], ra[b])

            colacc = accp.tile([128, M], F16, tag="colacc")
            rowh3s = rowp.tile([128, NT * MBLK], F16, tag="rowh3s")
            rowcon = accp.tile([128, NT], F32, tag="rowcon")

            for t in range(NT):
                t16 = evp.tile([128, M], F16)
                for g in range(NG):
                    p = ps.tile([128, G * MBLK], F32)
                    for j in range(G):
                        mb = g * G + j
                        nc.tensor.matmul(
                            p[:, j * MBLK:(j + 1) * MBLK],
                            la_sb[:, t * 128:(t + 1) * 128],
                            ra_sb[:, mb * MBLK:(mb + 1) * MBLK],
                            start=True, stop=True,
                        )
                    half = G * MBLK
                    nc.scalar.copy(t16[:, g * half:(g + 1) * half], p[:])
                    if t == 0:
                        # col-path init straight off PSUM on ScalarE
                        nc.scalar.copy(colacc[:, g * half:(g + 1) * half], p[:])

                # row path: max-tree 4096 -> 512 into the h3 store
                h1 = tre.tile([128, M // 2], F16, tag="h1")
                nc.vector.tensor_max(h1[:], t16[:, 0:2048], t16[:, 2048:4096])
                h2 = tre.tile([128, M // 4], F16, tag="h2")
                nc.vector.tensor_max(h2[:], h1[:, 0:1024], h1[:, 1024:2048])
                rsl = rowh3s[:, t * MBLK:(t + 1) * MBLK]
                nc.vector.tensor_max(rsl, h2[:, 0:512], h2[:, 512:1024])

                # col path: full-width accumulate
                if t > 0:
                    nc.vector.tensor_max(colacc[:], colacc[:], t16[:])

            # batch-end fold: rowh3s [128, NT, 512] -> rowcon [128, NT]
            v = rowh3s[:].rearrange("p (t w) -> p t w", w=MBLK)
            w = MBLK
            while w > 16:
                h = w // 2
                nc.vector.tensor_max(v[:, :, 0:h], v[:, :, 0:h], v[:, :, h:w])
                w = h
            nc.vector.tensor_reduce(rowcon[:], v[:, :, 0:w],
                                    axis=mybir.AxisListType.X, op=mx)

            nc.sync.dma_start(colaccs[b], colacc[:])
            nc.sync.dma_start(rowcons[b], rowcon[:])

        if repeat is not None:
            rep_cm.__exit__(None, None, None)

    nc.compile()
    return nc


def _get_nc():
    if "nc" not in _CACHE:
        _CACHE["nc"] = _build_nc()
    return _CACHE["nc"]


def _split16(x):
    hi = x.astype(np.float16)
    lo = (x.astype(np.float32) - hi.astype(np.float32)).astype(np.float16)
    return hi, lo


def _prepare(gts, preds):
    """Host prep: K=13 fp16 hi/lo augmented operands, per core."""
    gts = np.asarray(gts, dtype=np.float32)
    preds = np.asarray(preds, dtype=np.float32)
    assert gts.shape == (B, N, D) and preds.shape == (B, M, D)

    gh, gl = _split16(gts)                     # [B,N,3]
    ph = preds.astype(np.float16)
    g2 = np.einsum("bnd,bnd->bn", gts, gts)    # f32
    p2 = np.einsum("bmd,bmd->bm", preds, preds)
    g2h, g2l = _split16(g2)
    p2h, p2l = _split16(p2)

    la = np.empty((B, K, N), np.float16)
    ra = np.empty((B, K, M), np.float16)
    for d in range(D):
        la[:, 3 * d + 0] = gh[:, :, d]
        la[:, 3 * d + 1] = gh[:, :, d]
        la[:, 3 * d + 2] = gl[:, :, d]
        ra[:, 3 * d + 0] = (2.0 * ph[:, :, d].astype(np.float32)).astype(np.float16)
        ra[:, 3 * d + 1] = (2.0 * (preds[:, :, d] - ph[:, :, d].astype(np.float32))).astype(np.float16)
        ra[:, 3 * d + 2] = ra[:, 3 * d + 0]
    la[:, 9] = g2h
    la[:, 10] = g2l
    la[:, 11] = 1.0
    la[:, 12] = 1.0
    ra[:, 9] = -1.0
    ra[:, 10] = -1.0
    ra[:, 11] = -p2h
    ra[:, 12] = -p2l

    in_maps = []
    for c in range(N_CORES):
        sl = slice(c * BPC, (c + 1) * BPC)
        in_maps.append({
            "la": np.ascontiguousarray(la[sl]),
            "ra": np.ascontiguousarray(ra[sl]),
        })
    return in_maps


def _finalize(results):
    """Host fold: results[c] has colaccs [BPC,128,M] f16, rowcons [BPC,128,NT] f32."""
    col_sum = 0.0
    row_sum = 0.0
    for c in range(N_CORES):
        colaccs = np.asarray(results[c]["colaccs"], np.float32)  # [BPC,128,M]
        rowcons = np.asarray(results[c]["rowcons"], np.float32)  # [BPC,128,NT]
        # col: max over the 128 n-residues -> [BPC, M]; sum all
        col_sum += colaccs.max(axis=1).sum(dtype=np.float64)
        # row: already fully folded on device; sum all
        row_sum += rowcons.sum(dtype=np.float64)
    loss1 = -col_sum / (B * M)   # mean over (b,m) of min_n dist^2
    loss2 = -row_sum / (B * N)   # mean over (b,n) of min_m dist^2
    return np.float32(loss1 + loss2)


def _run(in_maps, trace=False):
    from concourse.bass_utils import run_bass_kernel_spmd
    nc = _get_nc()
    return run_bass_kernel_spmd(nc, in_maps, list(range(N_CORES)), trace=trace)


def kernel(gts, preds):
    in_maps = _prepare(gts, preds)
    res = _run(in_maps)
    return _finalize(res.results)


def run_profiled(gts, preds):
    """Like kernel() but asks for an NTFF trace; returns (loss, exec_time_ns, raw)."""
    in_maps = _prepare(gts, preds)
    res = _run(in_maps, trace=True)
    return _finalize(res.results), res.exec_time_ns, res
